# revision 19
# baseline (speedup 1.0000x reference)
"""Trainium2 Bass kernel for nn_E2EGuidedFilter (guided filter, r=8, eps=0.01).

Full inputs x, y: (8, 3, 1024, 1024) fp32. Data-parallel: one image per
NeuronCore (8 cores).

Per-core pipeline (per channel, H=W=1024, 8 partition-blocks of 128):
  - centered fp16 data: xc = f16(x-0.5), yc = f16(y-0.5) (host-prepped),
    plus a host-transposed copy xcb for the final combine.
  - stage-1 box means of xc, yc, xc*yc, xc^2 via TWO banded matmuls on the
    TensorEngine (data as the stationary operand, band+1/count folded into
    the moving weights): H-filter (layout A->B) then W-filter (B->A).
  - pointwise covariance algebra in layout A (centered => no catastrophic
    cancellation in fp16):
      num = mxy - mx*my ; den = (mxx + eps) - mx^2 ; a = num/den
      b' = (my + 0.5) - a*mx        (= b + 0.5a; the 0.5s cancel later)
  - stage-2 box filter of a, b': tensor_tensor_scan along W (free axis,
    recurrence S[t] = S[t-1] + d[t+8] - d[t-9]) + banded matmul over H
    (qh folded) -> layout B; final: out_B = qw * (S2a*xcb + S2b).
  - output is transposed (layout B); the host transposes back.
"""

import os
import sys

import numpy as np

for _p in ("/opt/trn_rl_repo", "/root/.axon_site/_ro/trn_rl_repo"):
    if os.path.isdir(_p) and _p not in sys.path:
        sys.path.append(_p)

R = 8
EPS = 0.01
H = W = 1024
PB = H // 128  # 8 partition blocks
C = 3
NCORES = 8
SCAN_LEN = W + R  # 1032
PAD0 = 18  # interior offset in scan input buffers (even -> 4B aligned)
BUF_W = PAD0 + W + 8 + 2  # 1052
D0_OFF = PAD0 - 17  # data1 AP offset; data0 offset = PAD0

_CACHE = {}


def _counts():
    i = np.arange(H)
    return (np.minimum(i + R, H - 1) - np.maximum(i - R, 0) + 1).astype(np.float64)


def _host_consts():
    nh = _counts()
    qh = (1.0 / nh).astype(np.float32)

    def band_block(c, lo, n):
        Wt = np.zeros((128, n), np.float32)
        for j in range(n):
            hp = lo + j
            k0 = max(0, hp - R - 128 * c)
            k1 = min(127, hp + R - 128 * c)
            if k0 <= k1:
                Wt[k0 : k1 + 1, j] = qh[hp]
        return Wt

    W0 = band_block(0, 0, 136)
    Wi = band_block(1, 120, 144)
    W7 = band_block(7, 888, 136)
    wq = np.concatenate([W0, Wi, W7], axis=1).astype(np.float16)  # [128,416]
    qv = qh.reshape(PB, 128).T.copy().astype(np.float32)  # [128,8]
    return wq, qv


def _mm_windows():
    halves = [[], []]
    for c in range(PB):
        lo = max(0, 128 * c - 8)
        hi = min(1024, 128 * c + 136)
        if c == 0:
            wt, wbase = "e0", 0
        elif c == PB - 1:
            wt, wbase = "e7", 888
        else:
            wt, wbase = "int", 128 * c - 8
        for hf in (0, 1):
            blo, bhi = 512 * hf, 512 * hf + 512
            s, e = max(lo, blo), min(hi, bhi)
            if s < e:
                halves[hf].append((c, s, e, wt, s - wbase, e - wbase))
    return halves


_HALVES = _mm_windows()


def _split_multi_waits(nc, mybir):
    """This container's walrus supports 1 sync wait per instruction (2 for
    EventSemaphore); Tile emits more. Move excess waits onto NoOps inserted
    just before the instruction on the same engine."""
    uid = [0]
    for f in nc.m.functions:
        for bb in f.blocks:
            out = []
            changed = False
            for inst in bb.instructions:
                si = inst.sync_info
                waits = list(si.on_wait) if si and si.on_wait else []
                cap = 2 if type(inst).__name__ == "InstEventSemaphore" else 1
                if len(waits) > cap:
                    for w in waits[:-cap]:
                        uid[0] += 1
                        nop = mybir.InstNoOp(name=f"wsplit-{uid[0]}", ins=[], outs=[])
                        nop.engine = inst.engine
                        nop.sync_info = mybir.SyncInfo(on_wait=[w], on_update=[])
                        out.append(nop)
                    si.on_wait = waits[-cap:]
                    changed = True
                out.append(inst)
            if changed:
                bb.instructions = out


def _build_bass():
    import concourse.bass as bass
    import concourse.mybir as mybir
    from concourse import tile
    from contextlib import ExitStack

    f16 = mybir.dt.float16
    f32 = mybir.dt.float32
    AF = mybir.ActivationFunctionType
    OP = mybir.AluOpType

    nc = bass.Bass("TRN2", target_bir_lowering=False, debug=False)

    xc_d = nc.dram_tensor("xc", [C, PB, 128, W], f16, kind="ExternalInput").ap()
    yc_d = nc.dram_tensor("yc", [C, PB, 128, W], f16, kind="ExternalInput").ap()
    xcb_d = nc.dram_tensor("xcb", [C, PB, 128, W], f16, kind="ExternalInput").ap()
    wq_d = nc.dram_tensor("wq", [128, 416], f16, kind="ExternalInput").ap()
    qv_d = nc.dram_tensor("qv", [128, PB], f32, kind="ExternalInput").ap()
    out_d = nc.dram_tensor("out", [C, PB, 128, W], f32, kind="ExternalOutput").ap()

    with tile.TileContext(nc) as tc, ExitStack() as ctx:
        pconst = ctx.enter_context(tc.tile_pool(name="const", bufs=1))
        wq_t = pconst.tile([128, 416], f16, tag="wq")
        nc.sync.dma_start(wq_t[:], wq_d[:])
        qv_t = pconst.tile([128, PB], f32, tag="qv")
        nc.sync.dma_start(qv_t[:], qv_d[:])

        def wslice(wt, a, b):
            if wt == "e0":
                return wq_t[:, a:b]
            if wt == "int":
                return wq_t[:, 136 + a : 136 + b]
            return wq_t[:, 280 + a : 280 + b]

        # ---- pools ----
        pin = {
            t: ctx.enter_context(tc.tile_pool(name=f"in_{t}", bufs=PB))
            for t in ("xc", "yc", "xy", "xx")
        }
        pmid = {
            t: ctx.enter_context(tc.tile_pool(name=f"mid_{t}", bufs=PB))
            for t in ("xc", "yc", "xy", "xx")
        }
        pab = ctx.enter_context(tc.tile_pool(name="ab_buf", bufs=2))
        pso2 = {
            t: ctx.enter_context(tc.tile_pool(name=f"so2_{t}", bufs=PB))
            for t in ("a", "b")
        }
        pz = ctx.enter_context(tc.tile_pool(name="z", bufs=8, space="PSUM"))
        ppw16 = ctx.enter_context(tc.tile_pool(name="pw16", bufs=3))
        ppw16c = ctx.enter_context(tc.tile_pool(name="pw16c", bufs=2))
        ppw32 = ctx.enter_context(tc.tile_pool(name="pw32", bufs=2))
        pxcb = ctx.enter_context(tc.tile_pool(name="xcb", bufs=1))
        pout = ctx.enter_context(tc.tile_pool(name="outst", bufs=1))

        def mm_group(z, hf, lhs_of):
            mms = [
                (z[:, s - 512 * hf : e - 512 * hf], lhs_of(c), wslice(wt, wa, wb))
                for c, s, e, wt, wa, wb in _HALVES[hf]
            ]
            for i, (o, l, r) in enumerate(mms):
                nc.tensor.matmul(
                    o, l, r,
                    start=(i == 0),
                    stop=(i == len(mms) - 1),
                    skip_group_check=True,
                )
            return z

        for ch in range(C):
            # ---- stage 0: load inputs, products (plain [128,1024] tiles) --
            tin = {t: [None] * PB for t in ("xc", "yc", "xy", "xx")}
            for b in range(PB):
                txc = pin["xc"].tile([128, W], f16, tag="xc")
                nc.sync.dma_start(txc[:], xc_d[ch, b])
                tyc = pin["yc"].tile([128, W], f16, tag="yc")
                nc.sync.dma_start(tyc[:], yc_d[ch, b])
                txy = pin["xy"].tile([128, W], f16, tag="xy")
                nc.vector.tensor_mul(txy[:], txc[:], tyc[:])
                txx = pin["xx"].tile([128, W], f16, tag="xx")
                nc.scalar.activation(txx[:], txc[:], AF.Square)
                tin["xc"][b] = txc
                tin["yc"][b] = tyc
                tin["xy"][b] = txy
                tin["xx"][b] = txx

            # ---- stage 1a: MM over H (A->B, qh folded), evac to mids ----
            mids = {t: [None] * PB for t in ("xc", "yc", "xy", "xx")}
            for m in range(PB):
                for t in ("xc", "yc", "xy", "xx"):
                    midt = pmid[t].tile([128, W], f16, tag=f"mid_{t}")
                    for hf in (0, 1):
                        zt = pz.tile([128, 512], f32, tag="z")
                        mm_group(
                            zt, hf,
                            lambda c, _t=t: tin[_t][c][:, 128 * m : 128 * m + 128],
                        )
                        if t in ("xc", "yc", "xy"):
                            nc.scalar.activation(
                                midt[:, 512 * hf : 512 * hf + 512], zt[:], AF.Copy
                            )
                        else:
                            nc.vector.tensor_copy(
                                midt[:, 512 * hf : 512 * hf + 512], zt[:]
                            )
                    mids[t][m] = midt

            # ---- stage 1b: MM over W (B->A, qw folded) + pointwise ----
            so2 = {"a": [None] * PB, "b": [None] * PB}
            for hc in range(PB):
                ba = pab.tile([128, BUF_W], f16, tag="a_buf")
                nc.gpsimd.memset(ba[:, 0:PAD0], 0.0)
                nc.gpsimd.memset(ba[:, PAD0 + W :], 0.0)
                bb = pab.tile([128, BUF_W], f16, tag="b_buf")
                nc.gpsimd.memset(bb[:, 0:PAD0], 0.0)
                nc.gpsimd.memset(bb[:, PAD0 + W :], 0.0)
                for hf in (0, 1):
                    z = {}
                    for t in ("xc", "yc", "xy", "xx"):
                        zt = pz.tile([128, 512], f32, tag="z")
                        mm_group(
                            zt, hf,
                            lambda m, _t=t: mids[_t][m][:, 128 * hc : 128 * hc + 128],
                        )
                        z[t] = zt
                    mx = ppw16.tile([128, 512], f16, tag="mx")
                    nc.scalar.activation(mx[:], z["xc"][:], AF.Copy)
                    my = ppw16.tile([128, 512], f16, tag="my")
                    nc.scalar.activation(my[:], z["yc"][:], AF.Copy)
                    t1 = ppw16.tile([128, 512], f16, tag="t1")
                    nc.vector.tensor_mul(t1[:], mx[:], my[:])
                    num = ppw16.tile([128, 512], f16, tag="num")
                    nc.vector.tensor_sub(num[:], z["xy"][:], t1[:])
                    s2 = ppw16c.tile([128, 512], f16, tag="s2")
                    nc.scalar.activation(s2[:], mx[:], AF.Square)
                    mxxe = ppw16c.tile([128, 512], f16, tag="mxxe")
                    nc.scalar.activation(mxxe[:], z["xx"][:], AF.Copy, bias=EPS)
                    den = ppw16c.tile([128, 512], f16, tag="den")
                    nc.vector.tensor_sub(den[:], mxxe[:], s2[:])
                    rinv = ppw32.tile([128, 512], f32, tag="rinv")
                    nc.vector.reciprocal(rinv[:], den[:])
                    av = ba[:, PAD0 + 512 * hf : PAD0 + 512 * hf + 512]
                    nc.vector.tensor_mul(av, num[:], rinv[:])
                    j1 = ppw16c.tile([128, 512], f16, tag="j1")
                    nc.vector.tensor_mul(j1[:], av, mx[:])
                    bv = bb[:, PAD0 + 512 * hf : PAD0 + 512 * hf + 512]
                    nc.vector.scalar_tensor_tensor(
                        bv, my[:], 0.5, j1[:], OP.add, OP.subtract
                    )
                # scans along W (box_W of a, b') right after this h-chunk
                for t, buf in (("a", ba), ("b", bb)):
                    so = pso2[t].tile([128, SCAN_LEN], f16, tag=f"so2_{t}")
                    nc.vector.tensor_tensor_scan(
                        so[:, :],
                        buf[:, PAD0 : PAD0 + SCAN_LEN],
                        buf[:, D0_OFF : D0_OFF + SCAN_LEN],
                        0.0,
                        OP.add,
                        OP.subtract,
                    )
                    so2[t][hc] = so

            for mw in range(PB):
                xcb_t = pxcb.tile([128, W], f16, tag="xcb")
                nc.sync.dma_start(xcb_t[:], xcb_d[ch, mw])
                ot = pout.tile([128, W], f32, tag="outst")
                for hf in (0, 1):
                    s2a = pz.tile([128, 512], f32, tag="z")
                    mm_group(
                        s2a, hf,
                        lambda c: so2["a"][c][:, 8 + 128 * mw : 8 + 128 * mw + 128],
                    )
                    s2b = pz.tile([128, 512], f32, tag="z")
                    mm_group(
                        s2b, hf,
                        lambda c: so2["b"][c][:, 8 + 128 * mw : 8 + 128 * mw + 128],
                    )
                    s2a_s = ppw16c.tile([128, 512], f16, tag="s2a_s")
                    nc.scalar.activation(s2a_s[:], s2a[:], AF.Copy)
                    f1 = ppw16c.tile([128, 512], f16, tag="f1")
                    nc.vector.tensor_mul(
                        f1[:], s2a_s[:], xcb_t[:, 512 * hf : 512 * hf + 512]
                    )
                    f2 = ppw32.tile([128, 512], f32, tag="f2")
                    nc.vector.tensor_add(f2[:], f1[:], s2b[:])
                    nc.scalar.activation(
                        ot[:, 512 * hf : 512 * hf + 512],
                        f2[:],
                        AF.Copy,
                        scale=qv_t[:, mw : mw + 1],
                    )
                nc.sync.dma_start(out_d[ch, mw], ot[:])

    _split_multi_waits(nc, mybir)
    return nc


def _get_bass():
    if "nc" not in _CACHE:
        _CACHE["nc"] = _build_bass()
    return _CACHE["nc"]


def kernel(x, y):
    x = np.asarray(x)
    y = np.asarray(y)
    from concourse.bass_utils import run_bass_kernel_spmd

    nc = _get_bass()
    wq, qv = _host_consts()
    B = x.shape[0]
    xcf = (x - 0.5).astype(np.float16)
    ycf = (y - 0.5).astype(np.float16)
    xc = xcf.reshape(B, C, PB, 128, W)
    yc = ycf.reshape(B, C, PB, 128, W)
    xcb = np.ascontiguousarray(xcf.transpose(0, 1, 3, 2)).reshape(B, C, PB, 128, W)
    in_maps = [
        {"xc": xc[i], "yc": yc[i], "xcb": xcb[i], "wq": wq, "qv": qv}
        for i in range(B)
    ]
    res = run_bass_kernel_spmd(nc, in_maps, core_ids=list(range(B)))
    out = np.stack(
        [
            res.results[i]["out"].reshape(C, W, H).transpose(0, 2, 1)
            for i in range(B)
        ]
    )
    return np.ascontiguousarray(out).astype(np.float32)


# revision 28
# speedup vs baseline: 1.0559x; 1.0559x over previous
"""Trainium2 Bass kernel for nn_E2EGuidedFilter (guided filter, r=8, eps=0.01).

Full inputs x, y: (8, 3, 1024, 1024) fp32. Data-parallel: one image per
NeuronCore (8 cores).

Per-core pipeline (per channel, H=W=1024, 8 partition-blocks of 128):
  - centered fp16 data: xc = f16(x-0.5), yc = f16(y-0.5) (host-prepped),
    plus a host-transposed copy xcb for the final combine.
  - stage-1 box means of xc, yc, xc*yc, xc^2 via TWO banded matmuls on the
    TensorEngine (data as the stationary operand, band+1/count folded into
    the moving weights): H-filter (layout A->B) then W-filter (B->A).
  - pointwise covariance algebra in layout A (centered => no catastrophic
    cancellation in fp16):
      num = mxy - mx*my ; den = (mxx + eps) - mx^2 ; a = num/den
      b' = (my + 0.5) - a*mx        (= b + 0.5a; the 0.5s cancel later)
  - stage-2 box filter of a, b': tensor_tensor_scan along W (free axis,
    recurrence S[t] = S[t-1] + d[t+8] - d[t-9]) + banded matmul over H
    (qh folded) -> layout B; final: out_B = qw * (S2a*xcb + S2b).
  - output is transposed (layout B); the host transposes back.
"""

import os
import sys

import numpy as np

for _p in ("/opt/trn_rl_repo", "/root/.axon_site/_ro/trn_rl_repo"):
    if os.path.isdir(_p) and _p not in sys.path:
        sys.path.append(_p)

R = 8
EPS = 0.01
H = W = 1024
PB = H // 128  # 8 partition blocks
C = 3
NCORES = 8
SCAN_LEN = W + R  # 1032
PAD0 = 18  # interior offset in scan input buffers (even -> 4B aligned)
BUF_W = PAD0 + W + 8 + 2  # 1052
D0_OFF = PAD0 - 17  # data1 AP offset; data0 offset = PAD0

_CACHE = {}


def _counts():
    i = np.arange(H)
    return (np.minimum(i + R, H - 1) - np.maximum(i - R, 0) + 1).astype(np.float64)


def _host_consts():
    nh = _counts()
    qh = (1.0 / nh).astype(np.float32)

    def band_block(c, lo, n):
        Wt = np.zeros((128, n), np.float32)
        for j in range(n):
            hp = lo + j
            k0 = max(0, hp - R - 128 * c)
            k1 = min(127, hp + R - 128 * c)
            if k0 <= k1:
                Wt[k0 : k1 + 1, j] = qh[hp]
        return Wt

    W0 = band_block(0, 0, 136)
    Wi = band_block(1, 120, 144)
    W7 = band_block(7, 888, 136)
    wq = np.concatenate([W0, Wi, W7], axis=1).astype(np.float16)  # [128,416]
    qv = qh.reshape(PB, 128).T.copy().astype(np.float32)  # [128,8]
    return wq, qv


def _mm_windows():
    halves = [[], []]
    for c in range(PB):
        lo = max(0, 128 * c - 8)
        hi = min(1024, 128 * c + 136)
        if c == 0:
            wt, wbase = "e0", 0
        elif c == PB - 1:
            wt, wbase = "e7", 888
        else:
            wt, wbase = "int", 128 * c - 8
        for hf in (0, 1):
            blo, bhi = 512 * hf, 512 * hf + 512
            s, e = max(lo, blo), min(hi, bhi)
            if s < e:
                halves[hf].append((c, s, e, wt, s - wbase, e - wbase))
    return halves


_HALVES = _mm_windows()


def _split_multi_waits(nc, mybir):
    """This container's walrus supports 1 sync wait per instruction (2 for
    EventSemaphore); Tile emits more. Move excess waits onto NoOps inserted
    just before the instruction on the same engine."""
    uid = [0]
    for f in nc.m.functions:
        for bb in f.blocks:
            out = []
            changed = False
            for inst in bb.instructions:
                si = inst.sync_info
                waits = list(si.on_wait) if si and si.on_wait else []
                cap = 2 if type(inst).__name__ == "InstEventSemaphore" else 1
                if len(waits) > cap:
                    for w in waits[:-cap]:
                        uid[0] += 1
                        nop = mybir.InstNoOp(name=f"wsplit-{uid[0]}", ins=[], outs=[])
                        nop.engine = inst.engine
                        nop.sync_info = mybir.SyncInfo(on_wait=[w], on_update=[])
                        out.append(nop)
                    si.on_wait = waits[-cap:]
                    changed = True
                out.append(inst)
            if changed:
                bb.instructions = out


def _build_bass():
    import concourse.bass as bass
    import concourse.mybir as mybir
    from concourse import tile
    from contextlib import ExitStack

    f16 = mybir.dt.float16
    f32 = mybir.dt.float32
    AF = mybir.ActivationFunctionType
    OP = mybir.AluOpType

    nc = bass.Bass("TRN2", target_bir_lowering=False, debug=False)

    xc_d = nc.dram_tensor("xc", [C, PB, 128, W], f16, kind="ExternalInput").ap()
    yc_d = nc.dram_tensor("yc", [C, PB, 128, W], f16, kind="ExternalInput").ap()
    xcb_d = nc.dram_tensor("xcb", [C, PB, 128, W], f16, kind="ExternalInput").ap()
    wq_d = nc.dram_tensor("wq", [128, 416], f16, kind="ExternalInput").ap()
    qv_d = nc.dram_tensor("qv", [128, PB], f32, kind="ExternalInput").ap()
    out_d = nc.dram_tensor("out", [C, PB, 128, W], f32, kind="ExternalOutput").ap()

    with tile.TileContext(nc) as tc, ExitStack() as ctx:
        pconst = ctx.enter_context(tc.tile_pool(name="const", bufs=1))
        wq_t = pconst.tile([128, 416], f16, tag="wq")
        nc.sync.dma_start(wq_t[:], wq_d[:])
        qv_t = pconst.tile([128, PB], f32, tag="qv")
        nc.sync.dma_start(qv_t[:], qv_d[:])

        def wslice(wt, a, b):
            if wt == "e0":
                return wq_t[:, a:b]
            if wt == "int":
                return wq_t[:, 136 + a : 136 + b]
            return wq_t[:, 280 + a : 280 + b]

        # ---- pools ----
        pin = {
            t: ctx.enter_context(tc.tile_pool(name=f"in_{t}", bufs=PB))
            for t in ("xc", "yc", "xy", "xx")
        }
        pmid = {
            t: ctx.enter_context(tc.tile_pool(name=f"mid_{t}", bufs=PB))
            for t in ("xc", "yc", "xy", "xx")
        }
        pab = ctx.enter_context(tc.tile_pool(name="ab_buf", bufs=2))
        pso2 = {
            t: ctx.enter_context(tc.tile_pool(name=f"so2_{t}", bufs=PB))
            for t in ("a", "b")
        }
        pz = ctx.enter_context(tc.tile_pool(name="z", bufs=4, space="PSUM"))
        ppw16 = ctx.enter_context(tc.tile_pool(name="pw16", bufs=2))
        ppw16c = ctx.enter_context(tc.tile_pool(name="pw16c", bufs=2, ))
        pxcb = ctx.enter_context(tc.tile_pool(name="xcb", bufs=1))
        pout = ctx.enter_context(tc.tile_pool(name="outst", bufs=1))

        def mm_group_full(z, lhs_of):
            mms = []
            for hf in (0, 1):
                first_in_bank = True
                for c, s, e, wt, wa, wb in _HALVES[hf]:
                    mms.append(
                        (z[:, s:e], lhs_of(c), wslice(wt, wa, wb), first_in_bank)
                    )
                    first_in_bank = False
            for i, (o, l, r, st) in enumerate(mms):
                nc.tensor.matmul(
                    o, l, r,
                    start=st,
                    stop=(i == len(mms) - 1),
                    skip_group_check=True,
                )
            return z

        for ch in range(C):
            # ---- stage 0: load inputs, products (plain [128,1024] tiles) --
            tin = {t: [None] * PB for t in ("xc", "yc", "xy", "xx")}
            for b in range(PB):
                txc = pin["xc"].tile([128, W], f16, tag="xc")
                nc.sync.dma_start(txc[:], xc_d[ch, b])
                tyc = pin["yc"].tile([128, W], f16, tag="yc")
                nc.sync.dma_start(tyc[:], yc_d[ch, b])
                txy = pin["xy"].tile([128, W], f16, tag="xy")
                nc.vector.tensor_mul(txy[:], txc[:], tyc[:])
                txx = pin["xx"].tile([128, W], f16, tag="xx")
                nc.scalar.activation(txx[:], txc[:], AF.Square)
                tin["xc"][b] = txc
                tin["yc"][b] = tyc
                tin["xy"][b] = txy
                tin["xx"][b] = txx

            # ---- stage 1a: MM over H (A->B, qh folded), evac to mids ----
            mids = {t: [None] * PB for t in ("xc", "yc", "xy", "xx")}
            for m in range(PB):
                for t in ("xc", "yc", "xy", "xx"):
                    midt = pmid[t].tile([128, W], f16, tag=f"mid_{t}")
                    zt = pz.tile([128, W], f32, tag="z")
                    mm_group_full(
                        zt,
                        lambda c, _t=t: tin[_t][c][:, 128 * m : 128 * m + 128],
                    )
                    if t in ("xc", "yc", "xy"):
                        nc.scalar.activation(midt[:], zt[:], AF.Copy)
                    else:
                        nc.vector.tensor_copy(midt[:], zt[:])
                    mids[t][m] = midt

            # ---- stage 1b: MM over W (B->A, qw folded) + pointwise ----
            so2 = {"a": [None] * PB, "b": [None] * PB}
            for hc in range(PB):
                ba = pab.tile([128, BUF_W], f16, tag="a_buf")
                nc.gpsimd.memset(ba[:, 0:PAD0], 0.0)
                nc.gpsimd.memset(ba[:, PAD0 + W :], 0.0)
                bb = pab.tile([128, BUF_W], f16, tag="b_buf")
                nc.gpsimd.memset(bb[:, 0:PAD0], 0.0)
                nc.gpsimd.memset(bb[:, PAD0 + W :], 0.0)
                z = {}
                for t in ("xc", "yc", "xy", "xx"):
                    zt = pz.tile([128, W], f32, tag="z")
                    mm_group_full(
                        zt,
                        lambda m, _t=t: mids[_t][m][:, 128 * hc : 128 * hc + 128],
                    )
                    z[t] = zt
                mx = ppw16.tile([128, W], f16, tag="mx")
                nc.scalar.activation(mx[:], z["xc"][:], AF.Copy)
                my = ppw16.tile([128, W], f16, tag="my")
                nc.scalar.activation(my[:], z["yc"][:], AF.Copy)
                t1 = ppw16.tile([128, W], f16, tag="t1")
                nc.vector.tensor_mul(t1[:], mx[:], my[:])
                num = ppw16.tile([128, W], f16, tag="num")
                nc.vector.tensor_sub(num[:], z["xy"][:], t1[:])
                s2 = ppw16.tile([128, W], f16, tag="t1")
                nc.scalar.activation(s2[:], mx[:], AF.Square)
                den = ppw16c.tile([128, W], f16, tag="den")
                nc.vector.scalar_tensor_tensor(
                    den[:], z["xx"][:], EPS, s2[:], OP.add, OP.subtract
                )
                rinv = ppw16c.tile([128, W], f16, tag="rinv")
                with nc.allow_low_precision(reason="18-bit reciprocal ample for eps-regularized den"):
                    nc.vector.reciprocal(rinv[:], den[:])
                av = ba[:, PAD0 : PAD0 + W]
                nc.vector.tensor_mul(av, num[:], rinv[:])
                j1 = ppw16c.tile([128, W], f16, tag="jx")
                nc.vector.tensor_mul(j1[:], av, mx[:])
                bv = bb[:, PAD0 : PAD0 + W]
                nc.vector.scalar_tensor_tensor(
                    bv, my[:], 0.5, j1[:], OP.add, OP.subtract
                )
                # scans along W (box_W of a, b') right after this h-chunk
                for t, buf in (("a", ba), ("b", bb)):
                    so = pso2[t].tile([128, SCAN_LEN], f16, tag=f"so2_{t}")
                    nc.vector.tensor_tensor_scan(
                        so[:, :],
                        buf[:, PAD0 : PAD0 + SCAN_LEN],
                        buf[:, D0_OFF : D0_OFF + SCAN_LEN],
                        0.0,
                        OP.add,
                        OP.subtract,
                    )
                    so2[t][hc] = so

            for mw in range(PB):
                xcb_t = pxcb.tile([128, W], f16, tag="xcb")
                nc.sync.dma_start(xcb_t[:], xcb_d[ch, mw])
                ot = pout.tile([128, W], f32, tag="outst")
                s2a = pz.tile([128, W], f32, tag="z")
                mm_group_full(
                    s2a,
                    lambda c: so2["a"][c][:, 8 + 128 * mw : 8 + 128 * mw + 128],
                )
                s2b = pz.tile([128, W], f32, tag="z")
                mm_group_full(
                    s2b,
                    lambda c: so2["b"][c][:, 8 + 128 * mw : 8 + 128 * mw + 128],
                )
                s2a_s = ppw16c.tile([128, W], f16, tag="fx")
                nc.scalar.activation(s2a_s[:], s2a[:], AF.Copy)
                f1 = ppw16c.tile([128, W], f16, tag="jx")
                nc.vector.tensor_mul(f1[:], s2a_s[:], xcb_t[:])
                f2 = ppw16c.tile([128, W], f16, tag="fx")
                nc.vector.tensor_add(f2[:], f1[:], s2b[:])
                nc.scalar.activation(
                    ot[:], f2[:], AF.Copy, scale=qv_t[:, mw : mw + 1]
                )
                nc.sync.dma_start(out_d[ch, mw], ot[:])

    _split_multi_waits(nc, mybir)
    return nc


def _get_bass():
    if "nc" not in _CACHE:
        _CACHE["nc"] = _build_bass()
    return _CACHE["nc"]


def kernel(x, y):
    x = np.asarray(x)
    y = np.asarray(y)
    from concourse.bass_utils import run_bass_kernel_spmd

    nc = _get_bass()
    wq, qv = _host_consts()
    B = x.shape[0]
    xcf = (x - 0.5).astype(np.float16)
    ycf = (y - 0.5).astype(np.float16)
    xc = xcf.reshape(B, C, PB, 128, W)
    yc = ycf.reshape(B, C, PB, 128, W)
    xcb = np.ascontiguousarray(xcf.transpose(0, 1, 3, 2)).reshape(B, C, PB, 128, W)
    in_maps = [
        {"xc": xc[i], "yc": yc[i], "xcb": xcb[i], "wq": wq, "qv": qv}
        for i in range(B)
    ]
    res = run_bass_kernel_spmd(nc, in_maps, core_ids=list(range(B)))
    out = np.stack(
        [
            res.results[i]["out"].reshape(C, W, H).transpose(0, 2, 1)
            for i in range(B)
        ]
    )
    return np.ascontiguousarray(out).astype(np.float32)
